# revision 23
# baseline (speedup 1.0000x reference)
"""Trainium2 Bass kernel for nn_BidPrefix (segment_reduce).

Reference semantics, per row r (B=65536 rows, S=512 cols):
    cp[k]    = prod(x[r, 0:k])                  (exclusive prefix product)
    survival = cp[bid]
    rate     = cp[mp] - cp[mp+1], or EPS when mp == 0
returned as (survival [B,1] f32, rate_last [B,1] f32).

Design: log-domain fused masked sums.
    ln cp[k] = sum_t (iota[t] < k) * ln(x[t])
The Activation engine computes L = ln(x + 1e-38) once (fp16).  Per
row-group on DVE:
  * scalar_tensor_tensor with fused add-accumulator:
        s_bid = sum((iota is_lt bid) * L) = ln cp[bid]; same for mp
  * tensor_mask_reduce with a width-1 window [mp, mp+1) and op=max is
    a true per-row gather of L[mp] (2x mode on fp16);
    cp[mp+1] = cp[mp] * e^{L[mp]} so no third masked sum is needed.
(The Pool engine cannot run TensorScalarPtr -- the ISA check rejects
it -- so all masked sums live on DVE.)
Epilogue: exp on ACT, rate = e^{s_mp} * (1 - e^{L[mp]}) * [mp != 0]
+ EPS * [mp == 0] (exact select).  All Ln activations precede all Exp
activations so each activation table loads once; a dummy Ln up front
hides the first load inside the DMA ramp.

Numerics: L >= ln(1e-38) = -87.5 (no inf/NaN), fp16 L carries <= 4.9e-4
relative error per element, so worst scale-relative output error is
~ max_k sqrt(k) e^-k * 5e-4 ~ 2e-4, far inside the 2e-2 gate.

Supertile: [128, G*512] with row r = i*128*G + p*G + g (partition-major:
each partition's DMA chunk is G*2KB contiguous).  The first supertiles
are split into smaller chunks so compute engines ramp up sooner.
bid_info is fetched in ONE DMA right after the first x chunk; outputs
are staged in SBUF and stored in one DMA per half.

Sharding: pure data parallel over the batch axis, B/8 = 8192 rows per
NeuronCore, same NEFF on all 8 cores (SPMD), outputs concatenated.
"""

import numpy as np

import concourse.bacc as bacc
import concourse.mybir as mybir
from concourse.tile import TileContext
from concourse.bass_utils import run_bass_kernel_spmd

f32 = mybir.dt.float32
f16 = mybir.dt.float16
i32 = mybir.dt.int32
Alu = mybir.AluOpType
Act = mybir.ActivationFunctionType

N_CORES = 8
B, S = 65536, 512
ROWS = B // N_CORES          # 8192 rows per core
G = 4                        # 512-wide row-groups per supertile
SUPER = 128 * G              # 512 rows per supertile
N_SUPER = ROWS // SUPER      # 16 supertiles per core
W = G * S
NK = N_SUPER * G             # 64 row-group columns per partition
EPS = 1e-7
NEG_BIG = -3.0e38            # accum_in seed for max-gather
HALF = NK // 2               # epilogue half split
# tensor_mask_reduce compiles but crashes the exec unit at runtime on this
# stack (bisect-verified), so the third masked sum uses an is_le STT instead.
USE_TMR = False


def build_bass():
    nc = bacc.Bacc()

    x = nc.dram_tensor("x", [ROWS, S], f32, kind="ExternalInput")
    bid_info = nc.dram_tensor("bid_info", [ROWS, 2], i32, kind="ExternalInput")
    surv_out = nc.dram_tensor("survival", [ROWS, 1], f32, kind="ExternalOutput")
    rate_out = nc.dram_tensor("rate_last", [ROWS, 1], f32, kind="ExternalOutput")

    x_v = x.rearrange("(i p g) s -> i p (g s)", p=128, g=G)
    bi_v = bid_info.rearrange("(i p g) c -> p i (g c)", p=128, g=G)
    so_v = surv_out.rearrange("(i p g) c -> p i (g c)", p=128, g=G)
    ro_v = rate_out.rearrange("(i p g) c -> p i (g c)", p=128, g=G)

    with TileContext(nc) as tc:
        with (
            tc.tile_pool(name="const", bufs=1) as cpool,
            tc.tile_pool(name="xbuf", bufs=4) as xpool,
            tc.tile_pool(name="lbuf", bufs=5) as lpool,
            tc.tile_pool(name="scr_d", bufs=2) as dpool,
        ):
            # ln bias (guards ln(0) -> -inf; 1e-38 leaves normal x unchanged)
            lnb = cpool.tile([128, 1], f32, tag="lnb")
            nc.vector.memset(lnb[:], 1e-38)

            # dummy Ln: pulls the natural_log table load into the DMA ramp
            warm = cpool.tile([128, 1], f32, tag="warm")
            nc.scalar.activation(out=warm[:], in_=lnb[:], func=Act.Ln)

            # fp16 iota 0..511 (integers <= 2048 are exact in fp16)
            it_i = cpool.tile([128, S], i32, tag="it_i")
            nc.gpsimd.iota(it_i[:], pattern=[[1, S]], base=0,
                           channel_multiplier=0)
            it_f = cpool.tile([128, S], f32, tag="it_f")
            nc.vector.tensor_copy(out=it_f[:], in_=it_i[:])
            it_h = cpool.tile([128, S], f16, tag="it_h")
            nc.vector.tensor_copy(out=it_h[:], in_=it_f[:])

            bi = cpool.tile([128, NK * 2], i32, tag="bi")
            bif = cpool.tile([128, NK * 2], f32, tag="bif")
            mp_pk = cpool.tile([128, NK], f32, tag="mp_pk")
            mpp1 = cpool.tile([128, NK], f32, tag="mpp1")
            m0 = cpool.tile([128, NK], f32, tag="m0")
            onem = cpool.tile([128, NK], f32, tag="onem")

            # masked log-sum accumulators; l_mp holds the gathered L[mp]
            # (USE_TMR) or the s_mp1 = ln cp[mp+1] masked sum (third STT)
            s_bid = cpool.tile([128, NK], f32, tag="s_bid")
            s_mp = cpool.tile([128, NK], f32, tag="s_mp")
            l_mp = cpool.tile([128, NK], f32, tag="l_mp")

            # output staging
            e_bid = cpool.tile([128, NK], f32, tag="e_bid")
            e_mp = cpool.tile([128, NK], f32, tag="e_mp")
            e_lmp = cpool.tile([128, NK], f32, tag="e_lmp")
            one_x = cpool.tile([128, NK], f32, tag="one_x")
            u_t = cpool.tile([128, NK], f32, tag="u_t")
            rate1 = cpool.tile([128, NK], f32, tag="rate1")
            rate_t = cpool.tile([128, NK], f32, tag="rate_t")

            def epilogue_half(h):
                lo, hi = h * HALF, (h + 1) * HALF
                sl = slice(lo, hi)
                # e_lmp leads: it depends on the last DVE producer, the
                # longest dependency chain; e_bid (surv) trails.
                nc.scalar.activation(out=e_lmp[:, sl], in_=l_mp[:, sl],
                                     func=Act.Exp)
                nc.scalar.activation(out=e_bid[:, sl], in_=s_bid[:, sl],
                                     func=Act.Exp)
                nc.scalar.activation(out=e_mp[:, sl], in_=s_mp[:, sl],
                                     func=Act.Exp)
                if USE_TMR:
                    # e_lmp = e^{L[mp]}: rate1 = e_mp*(1-e_lmp)*[mp!=0]
                    nc.scalar.activation(out=one_x[:, sl], in_=e_lmp[:, sl],
                                         func=Act.Copy, bias=1.0, scale=-1.0)
                    nc.vector.tensor_mul(out=u_t[:, sl], in0=one_x[:, sl],
                                         in1=onem[:, sl])
                    nc.vector.tensor_mul(out=rate1[:, sl], in0=e_mp[:, sl],
                                         in1=u_t[:, sl])
                else:
                    # e_lmp = cp[mp+1]: rate1 = (e_mp - e_lmp)*[mp!=0]
                    nc.vector.tensor_sub(out=u_t[:, sl], in0=e_mp[:, sl],
                                         in1=e_lmp[:, sl])
                    nc.vector.tensor_mul(out=rate1[:, sl], in0=u_t[:, sl],
                                         in1=onem[:, sl])
                nc.vector.scalar_tensor_tensor(
                    out=rate_t[:, sl], in0=m0[:, sl], scalar=EPS,
                    in1=rate1[:, sl], op0=Alu.mult, op1=Alu.add)
                ih = slice(h * (N_SUPER // 2), (h + 1) * (N_SUPER // 2))
                nc.sync.dma_start(
                    out=so_v[:, ih],
                    in_=e_bid[:, sl].rearrange("p (i g) -> p i g",
                                               i=N_SUPER // 2))
                nc.sync.dma_start(
                    out=ro_v[:, ih],
                    in_=rate_t[:, sl].rearrange("p (i g) -> p i g",
                                                i=N_SUPER // 2))

            # ramp: split the first supertiles into small chunks so the first
            # ln lands on ACT sooner and DVE doesn't starve at start.
            schedule = [(0, g, 1) for g in range(G)]
            schedule += [(1, 0, 2), (1, 2, 2)]
            schedule += [(i, 0, G) for i in range(2, N_SUPER)]

            first = True
            for (i, g0, gn) in schedule:
                wid = gn * S
                xt = xpool.tile([128, wid], f32, tag=f"xt{gn}")
                nc.sync.dma_start(out=xt[:],
                                  in_=x_v[i][:, g0 * S:(g0 + gn) * S])
                if first:
                    # bid_info DMA rides right behind the first x chunk
                    nc.sync.dma_start(
                        out=bi[:].rearrange("p (i k) -> p i k", i=N_SUPER),
                        in_=bi_v)

                lt = lpool.tile([128, wid], f16, tag=f"lt{gn}")
                nc.scalar.activation(out=lt[:], in_=xt[:], func=Act.Ln,
                                     bias=lnb[:])

                if first:
                    # bid/mp scalars for the STTs; DVE idles until the first
                    # ln lands, so this is free ramp time
                    nc.vector.tensor_copy(out=bif[:], in_=bi[:])
                    first = False
                elif (i, g0) == (1, 0):
                    # epilogue-only conversions, off the critical ramp
                    nc.vector.tensor_copy(
                        out=mp_pk[:],
                        in_=bif[:].rearrange("p (n c) -> p n c", c=2)[:, :, 0])
                    if USE_TMR:
                        nc.vector.tensor_scalar(out=mpp1[:], in0=mp_pk[:],
                                                scalar1=1.0, scalar2=None,
                                                op0=Alu.add)
                    nc.vector.tensor_scalar(out=m0[:], in0=mp_pk[:],
                                            scalar1=0.0, scalar2=None,
                                            op0=Alu.is_equal)
                    nc.vector.tensor_scalar(out=onem[:], in0=m0[:],
                                            scalar1=-1.0, scalar2=1.0,
                                            op0=Alu.mult, op1=Alu.add)

                # on the last supertile, emit all gathers first and the s_mp
                # sums last so the epilogue's longest dependency chains start
                # as early as possible while the final STTs still stream.
                if i == N_SUPER - 1:
                    phases = ("gather", "bid", "mp")
                else:
                    phases = ("all",)
                for phase in phases:
                    for g in range(g0, g0 + gn):
                        col = i * G + g
                        lg = lt[:, (g - g0) * S:(g - g0 + 1) * S]
                        mp_ap = bif[:, 2 * col:2 * col + 1]
                        bid_ap = bif[:, 2 * col + 1:2 * col + 2]

                        if phase in ("all", "gather"):
                            scrm = dpool.tile([128, S], f16, tag="scrm")
                            if USE_TMR:
                                # gather L[mp] via width-1 window max-reduce
                                nc.vector.tensor_mask_reduce(
                                    out=scrm[:], in_=lg,
                                    mask_start=mp_pk[:, col:col + 1],
                                    mask_end=mpp1[:, col:col + 1],
                                    scale=1.0, accum_in=NEG_BIG, op=Alu.max,
                                    accum_out=l_mp[:, col:col + 1])
                            else:
                                # s_mp1 = sum L[t], t <= mp  (= ln cp[mp+1])
                                nc.vector.scalar_tensor_tensor(
                                    out=scrm[:], in0=it_h[:], scalar=mp_ap,
                                    in1=lg, op0=Alu.is_le, op1=Alu.mult,
                                    accum_out=l_mp[:, col:col + 1])

                        if phase in ("all", "bid"):
                            scr = dpool.tile([128, S], f16, tag="scr")
                            nc.vector.scalar_tensor_tensor(
                                out=scr[:], in0=it_h[:], scalar=bid_ap, in1=lg,
                                op0=Alu.is_lt, op1=Alu.mult,
                                accum_out=s_bid[:, col:col + 1])

                        if phase in ("all", "mp"):
                            scr2 = dpool.tile([128, S], f16, tag="scr2")
                            nc.vector.scalar_tensor_tensor(
                                out=scr2[:], in0=it_h[:], scalar=mp_ap, in1=lg,
                                op0=Alu.is_lt, op1=Alu.mult,
                                accum_out=s_mp[:, col:col + 1])

            # both epilogue halves sit after every Ln in ACT program order:
            # one Exp table load, overlapped with the last supertiles' STTs.
            epilogue_half(0)
            epilogue_half(1)
    nc.finalize()
    return nc


_NC_CACHE = None


def _get_nc():
    global _NC_CACHE
    if _NC_CACHE is None:
        _NC_CACHE = build_bass()
    return _NC_CACHE


def kernel(x, bid_info):
    x = np.ascontiguousarray(np.asarray(x, dtype=np.float32))
    bid_info = np.ascontiguousarray(np.asarray(bid_info, dtype=np.int32))
    assert x.shape == (B, S) and bid_info.shape == (B, 2)

    nc = _get_nc()
    in_maps = [
        {
            "x": x[c * ROWS:(c + 1) * ROWS],
            "bid_info": bid_info[c * ROWS:(c + 1) * ROWS],
        }
        for c in range(N_CORES)
    ]
    res = run_bass_kernel_spmd(nc, in_maps, core_ids=list(range(N_CORES)))
    survival = np.concatenate([r["survival"] for r in res.results], axis=0)
    rate_last = np.concatenate([r["rate_last"] for r in res.results], axis=0)
    return survival, rate_last
